# revision 1
# baseline (speedup 1.0000x reference)
"""Trainium2 Bass kernel for 2-layer LSTM (H=16) time-series predictor.

Model (reference): x:[B,T] -> per-t scalar input into LSTMCell1(1->16) ->
LSTMCell2(16->16), teacher-forced over T steps, then head(h2)=fc2(leaky(fc1(h2)))
produces out[:,0]; 32 autoregressive steps feed head output back as input.
Output [B, 33].

Sharding: data-parallel over batch across 8 cores (256 batch each), weights
replicated. Per-core layout: hidden dim on partitions, batch on the free dim.

Hardware constraints that shape the design:
  * every vector/scalar instruction needs ALL operands on the same partition
    range -> every elementwise tensor (c, h, sigmoid/tanh outputs) lives on
    partition window 0:32 ([layer1; layer2] stacked), and the 4 gate types are
    separated along the PSUM *free* dim instead of partitions;
  * a matmul writes one [M<=32-aligned, N<=512] PSUM block -> 4 matmuls per
    step, one per gate type (i, f, o, g), each [32, 256];
  * layer2 lags one step so both layers' gates use the same h1 and one shared
    rhs: a rotating hx buffer [h1(16); h2(16); x_t(1); ones(1)], whose x/ones
    rows are DMA-prefilled straight from DRAM several steps ahead.
"""

import numpy as np

import concourse.bass as bass
import concourse.tile as tile
from concourse import bacc, mybir
from concourse.bass_utils import run_bass_kernel_spmd

F32 = mybir.dt.float32
AF = mybir.ActivationFunctionType

H = 16
B = 2048
T = 2048
FUT = 32
NCORES = 8
BC = B // NCORES  # 256 batch per core
NHX = 2           # rotation depth of the hx rhs buffers

# torch gate row order in the 4H weight matrices: i, f, g, o
_G = {"i": slice(0, H), "f": slice(H, 2 * H), "g": slice(2 * H, 3 * H), "o": slice(3 * H, 4 * H)}
# our gate order along the psum free dim / lhsT column blocks
_ORDER = ["i", "f", "o", "g"]


def _pack_weights(W_ih1, W_hh1, b_ih1, b_hh1, W_ih2, W_hh2, b_ih2, b_hh2,
                  fc1_w, fc1_b, fc2_w, fc2_b):
    b1 = b_ih1 + b_hh1  # [64]
    b2 = b_ih2 + b_hh2

    # main loop lhsTs; column block k (32 wide) = gate _ORDER[k], [l1(16)|l2(16)].
    # main_h rows = [h1(16); h2(16)]; main_x rows = [x(1); ones(1)].
    main_h = np.zeros((32, 128), np.float32)
    main_x = np.zeros((2, 128), np.float32)
    for k, gn in enumerate(_ORDER):
        c0 = 32 * k
        main_h[0:16, c0:c0 + 16] = W_hh1[_G[gn], :].T      # h1 -> layer1 gate
        main_h[0:16, c0 + 16:c0 + 32] = W_ih2[_G[gn], :].T  # h1 -> layer2 gate
        main_h[16:32, c0 + 16:c0 + 32] = W_hh2[_G[gn], :].T  # h2 -> layer2 gate
        main_x[0, c0:c0 + 16] = W_ih1[_G[gn], 0]            # x  -> layer1 gate
        main_x[1, c0:c0 + 16] = b1[_G[gn]]
        main_x[1, c0 + 16:c0 + 32] = b2[_G[gn]]

    # rollout layer1: split into K=1 part (input o) and K=17 part (h1 + bias)
    ro1x = np.zeros((1, 64), np.float32)   # columns: 4 gate blocks of 16
    ro1h = np.zeros((17, 64), np.float32)
    ro2h1 = np.zeros((17, 64), np.float32)  # h1 + bias part of layer2
    ro2h2 = np.zeros((16, 64), np.float32)  # h2 part of layer2
    for k, gn in enumerate(_ORDER):
        c0 = 16 * k
        ro1x[0, c0:c0 + 16] = W_ih1[_G[gn], 0]
        ro1h[0:16, c0:c0 + 16] = W_hh1[_G[gn], :].T
        ro1h[16, c0:c0 + 16] = b1[_G[gn]]
        ro2h1[0:16, c0:c0 + 16] = W_ih2[_G[gn], :].T
        ro2h1[16, c0:c0 + 16] = b2[_G[gn]]
        ro2h2[0:16, c0:c0 + 16] = W_hh2[_G[gn], :].T

    ro_fc1 = np.zeros((17, 8), np.float32)
    ro_fc1[0:16] = fc1_w.T  # fc1_w [8,16]
    ro_fc1[16] = fc1_b

    # M=1 matmuls misbehave on HW — replicate the fc2 column into M=8 and
    # read row 0 of the result instead
    ro_fc2 = np.zeros((9, 8), np.float32)
    ro_fc2[0:8] = fc2_w.T  # fc2_w [1,8]
    ro_fc2[8] = fc2_b

    return dict(main_h=main_h, main_x=main_x, ro1x=ro1x, ro1h=ro1h,
                ro2h1=ro2h1, ro2h2=ro2h2, ro_fc1=ro_fc1, ro_fc2=ro_fc2)


def _pack_x(x_core, t_steps):
    """x_core [BC, t] -> [t+1, 2, BC]: per step a [x_t; 1] pair (last x row 0)."""
    xe = np.ones((t_steps + 1, 2, x_core.shape[0]), np.float32)
    xe[:, 0, :] = 0.0
    xe[:t_steps, 0, :] = x_core.T.astype(np.float32)
    return xe


# ---------------------------------------------------------------------------
# device kernel
# ---------------------------------------------------------------------------

def _build(t_steps=T, fut=FUT, bc=BC, dbg=False, loop_steps=None):
    # loop_steps < t_steps runs fewer recurrence steps with identical I/O
    # sizes — for isolating device time via wall-clock deltas (output is
    # mathematically meaningless in that mode)
    nc = bacc.Bacc("TRN2", target_bir_lowering=False)
    dbg_d = {}
    if dbg == 3:
        for name, p in [("d_zt", 9), ("d_z", 8), ("d_ops", 8)]:
            dbg_d[name] = nc.dram_tensor(name, [p, bc], F32, kind="ExternalOutput")
        dbg_d["d_osb"] = nc.dram_tensor("d_osb", [1, fut + 1, bc], F32, kind="ExternalOutput")
    elif dbg:
        for name, p in [("d_h1e", 17), ("d_h2e", 17), ("d_rc1", 16), ("d_rc2", 16)]:
            dbg_d[name] = nc.dram_tensor(name, [p, bc], F32, kind="ExternalOutput")

    xe_d = nc.dram_tensor("xe", [t_steps + 1, 2, bc], F32, kind="ExternalInput")
    w_d = {}
    for name, shape in [("main_h", [32, 128]), ("main_x", [2, 128]),
                        ("ro1x", [1, 64]), ("ro1h", [17, 64]),
                        ("ro2h1", [17, 64]), ("ro2h2", [16, 64]),
                        ("ro_fc1", [17, 8]), ("ro_fc2", [9, 8])]:
        w_d[name] = nc.dram_tensor(name, shape, F32, kind="ExternalInput")
    out_d = nc.dram_tensor("out", [fut + 1, bc], F32, kind="ExternalOutput")

    with tile.TileContext(nc) as tc:
        consts = tc.alloc_tile_pool(name="consts", bufs=1)
        states = tc.alloc_tile_pool(name="states", bufs=1)
        work = tc.alloc_tile_pool(name="work", bufs=3)
        xst = tc.alloc_tile_pool(name="xst", bufs=8)
        # main psum (4 banks) and rollout psum (4 banks) stay disjoint for the
        # whole kernel: recycling banks across pools while late main-loop ACT
        # reads are in flight corrupts results (PE-write/engine-read same-bank
        # hazard)
        psum = tc.alloc_tile_pool(name="psum", bufs=2, space="PSUM")
        psro = tc.alloc_tile_pool(name="psro", bufs=1, space="PSUM")

        w_sb = {}
        for name, t_d in w_d.items():
            w_sb[name] = consts.tile(list(t_d.shape), F32, tag=name, name=name)
            nc.sync.dma_start(out=w_sb[name], in_=t_d[:])

        # rotating rhs buffers: [h1(0:16); h2(16:32)]
        hx = []
        for q in range(NHX):
            hq = states.tile([32, bc], F32, tag=f"hx{q}", name=f"hx{q}")
            nc.vector.memset(hq, 0.0)
            hx.append(hq)
        cc = states.tile([32, bc], F32, tag="cc")   # [c1; c2]
        nc.vector.memset(cc, 0.0)

        wmh, wmx = w_sb["main_h"], w_sb["main_x"]

        # rollout state tiles (declared early; layer-1 snapshots are taken
        # between main-loop steps T-1 and T)
        h1e = states.tile([17, bc], F32, tag="h1e")  # h1 | ones
        h2e = states.tile([17, bc], F32, tag="h2e")  # h2 | ones
        rc1 = states.tile([16, bc], F32, tag="rc1")
        rc2 = states.tile([16, bc], F32, tag="rc2")

        # ---------------- main teacher-forced loop ----------------
        def body(j):
            cur = hx[j % NHX]
            nxt = hx[(j + 1) % NHX]
            xs = xst.tile([2, bc], F32, tag="xs")
            nc.sync.dma_start(out=xs, in_=xe_d[j])

            g = psum.tile([32, 4, bc], F32, tag="g")  # free: gate-type x batch
            for k in range(4):
                # x+bias then h, closing each accumulation group before the
                # next opens (concurrent groups in one psum zero region are
                # illegal)
                nc.tensor.matmul(g[:, k, :], wmx[:, 32 * k:32 * k + 32], xs,
                                 start=True, stop=False)
                nc.tensor.matmul(g[:, k, :], wmh[:, 32 * k:32 * k + 32], cur,
                                 start=False, stop=True)

            sif = work.tile([32, 3, bc], F32, tag="sif")
            nc.scalar.activation(sif, g[:, 0:3, :], AF.Sigmoid)
            tg = work.tile([32, bc], F32, tag="tg")
            nc.scalar.activation(tg, g[:, 3, :], AF.Tanh)

            # j==0: layer-1 half only (layer-2 gates are not yet valid).
            # j==t_steps: full window (base-16 slices are illegal); the
            # layer-1 results of this step are junk but harmless — rc1/h1e
            # snapshot c1(T-1)/h1(T-1) before this step's writes land.
            s0, s1 = (0, 16) if j == 0 else (0, 32)
            m1 = work.tile([32, bc], F32, tag="m1")
            m2 = work.tile([32, bc], F32, tag="m2")
            tc_ = work.tile([32, bc], F32, tag="tc")
            nc.vector.tensor_mul(m1[s0:s1], sif[s0:s1, 1, :], cc[s0:s1])
            nc.vector.tensor_mul(m2[s0:s1], sif[s0:s1, 0, :], tg[s0:s1])
            nc.vector.tensor_add(cc[s0:s1], m1[s0:s1], m2[s0:s1])
            nc.scalar.activation(tc_[s0:s1], cc[s0:s1], AF.Tanh)
            nc.vector.tensor_mul(nxt[s0:s1], sif[s0:s1, 2, :], tc_[s0:s1])

        n_loop = t_steps if loop_steps is None else loop_steps
        for j in range(n_loop):
            body(j)
        # snapshot layer-1 state before the final (layer-2-only) step clobbers it
        nc.scalar.copy(h1e[0:16], hx[n_loop % NHX][0:16])   # h1(T-1)
        nc.scalar.copy(rc1, cc[0:16])                        # c1(T-1)
        body(n_loop)

        # ---------------- rollout ----------------
        ot = states.tile([1, bc], F32, tag="ot")     # current head output
        zt = states.tile([9, bc], F32, tag="zt")     # leaky(fc1) | ones
        out_sb = states.tile([1, fut + 1, bc], F32, tag="out_sb")
        # ones rows (memset can't start at partition 16/8 — DMA from xe ones row)
        nc.sync.dma_start(out=h1e[16:17, :], in_=xe_d[n_loop, 1:2])
        nc.sync.dma_start(out=h2e[16:17, :], in_=xe_d[n_loop, 1:2])
        nc.sync.dma_start(out=zt[8:9, :], in_=xe_d[n_loop, 1:2])

        nc.sync.dma_start(out=h2e[0:16, :], in_=hx[(n_loop + 1) % NHX][16:32, :])  # h2(T-1), repartition
        nc.sync.dma_start(out=rc2[:], in_=cc[16:32, :])

        if dbg == 1:
            for name, t in [("d_h1e", h1e), ("d_h2e", h2e), ("d_rc1", rc1), ("d_rc2", rc2)]:
                nc.sync.dma_start(out=dbg_d[name][:], in_=t[:])

        last_ops = []

        def head(r):
            z = psro.tile([8, bc], F32, tag="roz")
            nc.tensor.matmul(z, w_sb["ro_fc1"], h2e, start=True, stop=True)
            zs = work.tile([8, bc], F32, tag="zs")
            nc.scalar.mul(zs, z, 0.2)
            nc.vector.tensor_max(zt[0:8], z, zs)  # leaky relu 0.2
            o_ps = psro.tile([8, bc], F32, tag="roo")
            nc.tensor.matmul(o_ps, w_sb["ro_fc2"], zt, start=True, stop=True)
            last_ops[:] = [o_ps]
            nc.scalar.copy(out_sb[:, r, :], o_ps[0:1])
            if r <= fut - 1:
                nc.scalar.copy(ot, o_ps[0:1])

        def ro_cell(mms, rc, h_out):
            gr = psro.tile([16, 4, bc], F32, tag="rog")
            for k in range(4):
                for i, (lhsT, rhs) in enumerate(mms):
                    nc.tensor.matmul(gr[:, k, :], lhsT[:, 16 * k:16 * k + 16], rhs,
                                     start=(i == 0), stop=(i == len(mms) - 1))
            sifr = work.tile([16, 3, bc], F32, tag="sifr")
            nc.scalar.activation(sifr, gr[:, 0:3, :], AF.Sigmoid)
            tgr = work.tile([16, bc], F32, tag="tgr")
            nc.scalar.activation(tgr, gr[:, 3, :], AF.Tanh)
            a1 = work.tile([16, bc], F32, tag="a1")
            a2 = work.tile([16, bc], F32, tag="a2")
            tcr = work.tile([16, bc], F32, tag="tcr")
            nc.vector.tensor_mul(a1, sifr[:, 1, :], rc)
            nc.vector.tensor_mul(a2, sifr[:, 0, :], tgr)
            nc.vector.tensor_add(rc, a1, a2)
            nc.scalar.activation(tcr, rc, AF.Tanh)
            nc.vector.tensor_mul(h_out, sifr[:, 2, :], tcr)

        head(0)
        for r in range(fut):
            ro_cell([(w_sb["ro1x"], ot), (w_sb["ro1h"], h1e)], rc1, h1e[0:16])
            ro_cell([(w_sb["ro2h1"], h1e), (w_sb["ro2h2"], h2e[0:16])], rc2, h2e[0:16])
            head(r + 1)

        if dbg == 2:
            for name, t in [("d_h1e", h1e), ("d_h2e", h2e), ("d_rc1", rc1), ("d_rc2", rc2)]:
                nc.sync.dma_start(out=dbg_d[name][:], in_=t[:])
        if dbg == 3:
            nc.sync.dma_start(out=dbg_d["d_zt"][:], in_=zt[:])
            ops_sb = states.tile([8, bc], F32, tag="ops_sb")
            nc.scalar.copy(ops_sb, last_ops[0])
            nc.sync.dma_start(out=dbg_d["d_ops"][:], in_=ops_sb[:])
            nc.sync.dma_start(out=dbg_d["d_osb"][:], in_=out_sb[:])
            z2 = psro.tile([8, bc], F32, tag="roz")
            nc.tensor.matmul(z2, w_sb["ro_fc1"], h2e, start=True, stop=True)
            z2s = states.tile([8, bc], F32, tag="z2s")
            nc.scalar.copy(z2s, z2)
            nc.sync.dma_start(out=dbg_d["d_z"][:], in_=z2s[:])

        # keep the partition dim in the AP — integer-indexing it away breaks
        # Tile's subtile dependency tracking (the DMA then reads stale data)
        nc.sync.dma_start(out=out_d[:].rearrange("(o f) b -> o f b", o=1), in_=out_sb)

        for p_ in (psro, psum, xst, work, states, consts):
            p_.release()

    if not nc.is_finalized():
        nc.finalize()
    return nc


_CACHED = {}


def _get_nc(t_steps, fut, bc, loop_steps=None):
    key = (t_steps, fut, bc, loop_steps)
    if key not in _CACHED:
        _CACHED[key] = _build(t_steps, fut, bc, loop_steps=loop_steps)
    return _CACHED[key]


def kernel(x, W_ih1, W_hh1, b_ih1, b_hh1, W_ih2, W_hh2, b_ih2, b_hh2,
           fc1_w, fc1_b, fc2_w, fc2_b, future, _t_steps=None, _trace=False,
           _loop_steps=None):
    x = np.asarray(x, np.float32)
    fut = int(future)
    t_steps = int(_t_steps or x.shape[1])
    bc = x.shape[0] // NCORES

    w = _pack_weights(np.asarray(W_ih1, np.float32), np.asarray(W_hh1, np.float32),
                      np.asarray(b_ih1, np.float32), np.asarray(b_hh1, np.float32),
                      np.asarray(W_ih2, np.float32), np.asarray(W_hh2, np.float32),
                      np.asarray(b_ih2, np.float32), np.asarray(b_hh2, np.float32),
                      np.asarray(fc1_w, np.float32), np.asarray(fc1_b, np.float32),
                      np.asarray(fc2_w, np.float32), np.asarray(fc2_b, np.float32))

    nc = _get_nc(t_steps, fut, bc, _loop_steps)
    in_maps = []
    for c in range(NCORES):
        m = dict(w)
        m["xe"] = _pack_x(x[c * bc : (c + 1) * bc, :t_steps], t_steps)
        in_maps.append(m)

    res = run_bass_kernel_spmd(nc, in_maps, core_ids=list(range(NCORES)), trace=_trace)
    outs = [res.results[c]["out"] for c in range(NCORES)]  # each [fut+1, bc]
    full = np.concatenate(outs, axis=1).T  # [B, fut+1]
    kernel._last_exec_ns = res.exec_time_ns
    return np.ascontiguousarray(full.astype(np.float32))



# revision 2
# speedup vs baseline: 67.2965x; 67.2965x over previous
"""Trainium2 Bass kernel for 2-layer LSTM (H=16) time-series predictor.

Model (reference): x:[B,T] -> per-t scalar input into LSTMCell1(1->16) ->
LSTMCell2(16->16), teacher-forced over T steps, then head(h2)=fc2(leaky(fc1(h2)))
produces out[:,0]; 32 autoregressive steps feed head output back as input.
Output [B, 33].

Sharding: data-parallel over batch across 8 cores (256 batch each), weights
replicated. Per-core layout: hidden dim on partitions, batch on the free dim.

Hardware constraints that shape the design:
  * every vector/scalar instruction needs ALL operands on the same partition
    range -> every elementwise tensor (c, h, sigmoid/tanh outputs) lives on
    partition window 0:32 ([layer1; layer2] stacked), and the 4 gate types are
    separated along the PSUM *free* dim instead of partitions;
  * a matmul writes one [M<=32-aligned, N<=512] PSUM block -> 4 matmuls per
    step, one per gate type (i, f, o, g), each [32, 256];
  * layer2 lags one step so both layers' gates use the same h1 and one shared
    rhs: a rotating hx buffer [h1(16); h2(16); x_t(1); ones(1)], whose x/ones
    rows are DMA-prefilled straight from DRAM several steps ahead.

The T-step recurrence runs in a HARDWARE loop (tc.For_i, UNROLL sub-steps per
iteration) instead of being fully unrolled: a fully unrolled 2048-step body is
~33k instructions, and both NEFF load time and instruction streaming scale
with that; the hw loop keeps the NEFF ~1k instructions regardless of T.
The first UNROLL steps are peeled (static) so step 0's half-window special
case stays out of the loop; the final step is peeled so the layer-1 state
snapshot for the rollout can happen between steps T-1 and T.
"""

import numpy as np

import concourse.bass as bass
import concourse.tile as tile
from concourse import bacc, mybir
from concourse.bass import ds
from concourse.bass_utils import run_bass_kernel_spmd

F32 = mybir.dt.float32
AF = mybir.ActivationFunctionType

H = 16
B = 2048
T = 2048
FUT = 32
NCORES = 8
BC = B // NCORES  # 256 batch per core
NHX = 2           # rotation depth of the hx rhs buffers
UNROLL = 16       # recurrence steps per hw-loop iteration

# torch gate row order in the 4H weight matrices: i, f, g, o
_G = {"i": slice(0, H), "f": slice(H, 2 * H), "g": slice(2 * H, 3 * H), "o": slice(3 * H, 4 * H)}
# our gate order along the psum free dim / lhsT column blocks
_ORDER = ["i", "f", "o", "g"]


def _pack_weights(W_ih1, W_hh1, b_ih1, b_hh1, W_ih2, W_hh2, b_ih2, b_hh2,
                  fc1_w, fc1_b, fc2_w, fc2_b):
    b1 = b_ih1 + b_hh1  # [64]
    b2 = b_ih2 + b_hh2

    # main loop lhsTs; column block k (32 wide) = gate _ORDER[k], [l1(16)|l2(16)].
    # main_h rows = [h1(16); h2(16)]; main_x rows = [x(1); ones(1)].
    main_h = np.zeros((32, 128), np.float32)
    main_x = np.zeros((2, 128), np.float32)
    for k, gn in enumerate(_ORDER):
        c0 = 32 * k
        main_h[0:16, c0:c0 + 16] = W_hh1[_G[gn], :].T      # h1 -> layer1 gate
        main_h[0:16, c0 + 16:c0 + 32] = W_ih2[_G[gn], :].T  # h1 -> layer2 gate
        main_h[16:32, c0 + 16:c0 + 32] = W_hh2[_G[gn], :].T  # h2 -> layer2 gate
        main_x[0, c0:c0 + 16] = W_ih1[_G[gn], 0]            # x  -> layer1 gate
        main_x[1, c0:c0 + 16] = b1[_G[gn]]
        main_x[1, c0 + 16:c0 + 32] = b2[_G[gn]]

    # rollout layer1: split into K=1 part (input o) and K=17 part (h1 + bias)
    ro1x = np.zeros((1, 64), np.float32)   # columns: 4 gate blocks of 16
    ro1h = np.zeros((17, 64), np.float32)
    ro2h1 = np.zeros((17, 64), np.float32)  # h1 + bias part of layer2
    ro2h2 = np.zeros((16, 64), np.float32)  # h2 part of layer2
    for k, gn in enumerate(_ORDER):
        c0 = 16 * k
        ro1x[0, c0:c0 + 16] = W_ih1[_G[gn], 0]
        ro1h[0:16, c0:c0 + 16] = W_hh1[_G[gn], :].T
        ro1h[16, c0:c0 + 16] = b1[_G[gn]]
        ro2h1[0:16, c0:c0 + 16] = W_ih2[_G[gn], :].T
        ro2h1[16, c0:c0 + 16] = b2[_G[gn]]
        ro2h2[0:16, c0:c0 + 16] = W_hh2[_G[gn], :].T

    ro_fc1 = np.zeros((17, 8), np.float32)
    ro_fc1[0:16] = fc1_w.T  # fc1_w [8,16]
    ro_fc1[16] = fc1_b

    # M=1 matmuls misbehave on HW — replicate the fc2 column into M=8 and
    # read row 0 of the result instead
    ro_fc2 = np.zeros((9, 8), np.float32)
    ro_fc2[0:8] = fc2_w.T  # fc2_w [1,8]
    ro_fc2[8] = fc2_b

    return dict(main_h=main_h, main_x=main_x, ro1x=ro1x, ro1h=ro1h,
                ro2h1=ro2h1, ro2h2=ro2h2, ro_fc1=ro_fc1, ro_fc2=ro_fc2)


def _pack_x(x_core, t_steps):
    """x_core [BC, t] -> [t+1, 2, BC]: per step a [x_t; 1] pair (last x row 0)."""
    xe = np.ones((t_steps + 1, 2, x_core.shape[0]), np.float32)
    xe[:, 0, :] = 0.0
    xe[:t_steps, 0, :] = x_core.T.astype(np.float32)
    return xe


# ---------------------------------------------------------------------------
# device kernel
# ---------------------------------------------------------------------------

def _build(t_steps=T, fut=FUT, bc=BC, dbg=False, loop_steps=None):
    # loop_steps < t_steps runs fewer recurrence steps with identical I/O
    # sizes — for isolating device time via wall-clock deltas (output is
    # mathematically meaningless in that mode)
    nc = bacc.Bacc("TRN2", target_bir_lowering=False)
    dbg_d = {}
    if dbg == 3:
        for name, p in [("d_zt", 9), ("d_z", 8), ("d_ops", 8)]:
            dbg_d[name] = nc.dram_tensor(name, [p, bc], F32, kind="ExternalOutput")
        dbg_d["d_osb"] = nc.dram_tensor("d_osb", [1, fut + 1, bc], F32, kind="ExternalOutput")
    elif dbg:
        for name, p in [("d_h1e", 17), ("d_h2e", 17), ("d_rc1", 16), ("d_rc2", 16)]:
            dbg_d[name] = nc.dram_tensor(name, [p, bc], F32, kind="ExternalOutput")

    xe_d = nc.dram_tensor("xe", [t_steps + 1, 2, bc], F32, kind="ExternalInput")
    w_d = {}
    for name, shape in [("main_h", [32, 128]), ("main_x", [2, 128]),
                        ("ro1x", [1, 64]), ("ro1h", [17, 64]),
                        ("ro2h1", [17, 64]), ("ro2h2", [16, 64]),
                        ("ro_fc1", [17, 8]), ("ro_fc2", [9, 8])]:
        w_d[name] = nc.dram_tensor(name, shape, F32, kind="ExternalInput")
    out_d = nc.dram_tensor("out", [fut + 1, bc], F32, kind="ExternalOutput")

    with tile.TileContext(nc) as tc:
        consts = tc.alloc_tile_pool(name="consts", bufs=1)
        states = tc.alloc_tile_pool(name="states", bufs=1)
        work = tc.alloc_tile_pool(name="work", bufs=3)
        xst = tc.alloc_tile_pool(name="xst", bufs=UNROLL)
        # main psum (4 banks) and rollout psum (4 banks) stay disjoint for the
        # whole kernel: recycling banks across pools while late main-loop ACT
        # reads are in flight corrupts results (PE-write/engine-read same-bank
        # hazard)
        psum = tc.alloc_tile_pool(name="psum", bufs=2, space="PSUM")
        psro = tc.alloc_tile_pool(name="psro", bufs=1, space="PSUM")

        w_sb = {}
        for name, t_d in w_d.items():
            w_sb[name] = consts.tile(list(t_d.shape), F32, tag=name, name=name)
            nc.sync.dma_start(out=w_sb[name], in_=t_d[:])

        # rotating rhs buffers: [h1(0:16); h2(16:32)]
        hx = []
        for q in range(NHX):
            hq = states.tile([32, bc], F32, tag=f"hx{q}", name=f"hx{q}")
            nc.vector.memset(hq, 0.0)
            hx.append(hq)
        cc = states.tile([32, bc], F32, tag="cc")   # [c1; c2]
        nc.vector.memset(cc, 0.0)

        wmh, wmx = w_sb["main_h"], w_sb["main_x"]

        # rollout state tiles (declared early; layer-1 snapshots are taken
        # between main-loop steps T-1 and T)
        h1e = states.tile([17, bc], F32, tag="h1e")  # h1 | ones
        h2e = states.tile([17, bc], F32, tag="h2e")  # h2 | ones
        rc1 = states.tile([16, bc], F32, tag="rc1")
        rc2 = states.tile([16, bc], F32, tag="rc2")

        # ---------------- main teacher-forced loop ----------------
        def body(j_ap, parity, first=False, last=False):
            # j_ap: int or IV expression indexing xe_d; parity: static j%NHX
            cur = hx[parity % NHX]
            nxt = hx[(parity + 1) % NHX]
            xs = xst.tile([2, bc], F32, tag="xs")
            if isinstance(j_ap, int):
                nc.sync.dma_start(out=xs, in_=xe_d[j_ap])
            else:
                nc.sync.dma_start(
                    out=xs, in_=xe_d[ds(j_ap, 1)].rearrange("o p b -> (o p) b"))

            g = psum.tile([32, 4, bc], F32, tag="g")  # free: gate-type x batch
            for k in range(4):
                # x+bias then h, closing each accumulation group before the
                # next opens (concurrent groups in one psum zero region are
                # illegal)
                nc.tensor.matmul(g[:, k, :], wmx[:, 32 * k:32 * k + 32], xs,
                                 start=True, stop=False)
                nc.tensor.matmul(g[:, k, :], wmh[:, 32 * k:32 * k + 32], cur,
                                 start=False, stop=True)

            sif = work.tile([32, 3, bc], F32, tag="sif")
            nc.scalar.activation(sif, g[:, 0:3, :], AF.Sigmoid)
            tg = work.tile([32, bc], F32, tag="tg")
            nc.scalar.activation(tg, g[:, 3, :], AF.Tanh)

            # first step: layer-1 half only (layer-2 gates are not yet valid).
            # last step: full window (base-16 slices are illegal); the
            # layer-1 results of this step are junk but harmless — rc1/h1e
            # snapshot c1(T-1)/h1(T-1) before this step's writes land.
            s0, s1 = (0, 16) if first else (0, 32)
            m1 = work.tile([32, bc], F32, tag="m1")
            m2 = work.tile([32, bc], F32, tag="m2")
            tc_ = work.tile([32, bc], F32, tag="tc")
            nc.vector.tensor_mul(m1[s0:s1], sif[s0:s1, 1, :], cc[s0:s1])
            nc.vector.tensor_mul(m2[s0:s1], sif[s0:s1, 0, :], tg[s0:s1])
            nc.vector.tensor_add(cc[s0:s1], m1[s0:s1], m2[s0:s1])
            nc.scalar.activation(tc_[s0:s1], cc[s0:s1], AF.Tanh)
            nc.vector.tensor_mul(nxt[s0:s1], sif[s0:s1, 2, :], tc_[s0:s1])

        n_loop = t_steps if loop_steps is None else loop_steps
        assert n_loop >= UNROLL and (n_loop - UNROLL) % UNROLL == 0, n_loop
        # peel the first UNROLL steps (covers the j==0 special case)
        for j in range(UNROLL):
            body(j, j % NHX, first=(j == 0))
        # hardware loop over the middle steps, UNROLL at a time
        if n_loop > UNROLL:
            with tc.For_i(UNROLL, n_loop, UNROLL) as jb:
                for u in range(UNROLL):
                    body(jb + u, u % NHX)
        # snapshot layer-1 state before the final (layer-2-only) step clobbers it
        nc.scalar.copy(h1e[0:16], hx[n_loop % NHX][0:16])   # h1(T-1)
        nc.scalar.copy(rc1, cc[0:16])                        # c1(T-1)
        body(n_loop, n_loop % NHX, last=True)

        # ---------------- rollout ----------------
        ot = states.tile([1, bc], F32, tag="ot")     # current head output
        zt = states.tile([9, bc], F32, tag="zt")     # leaky(fc1) | ones
        out_sb = states.tile([1, fut + 1, bc], F32, tag="out_sb")
        # ones rows (memset can't start at partition 16/8 — DMA from xe ones row)
        nc.sync.dma_start(out=h1e[16:17, :], in_=xe_d[n_loop, 1:2])
        nc.sync.dma_start(out=h2e[16:17, :], in_=xe_d[n_loop, 1:2])
        nc.sync.dma_start(out=zt[8:9, :], in_=xe_d[n_loop, 1:2])

        nc.sync.dma_start(out=h2e[0:16, :], in_=hx[(n_loop + 1) % NHX][16:32, :])  # h2(T-1), repartition
        nc.sync.dma_start(out=rc2[:], in_=cc[16:32, :])

        if dbg == 1:
            for name, t in [("d_h1e", h1e), ("d_h2e", h2e), ("d_rc1", rc1), ("d_rc2", rc2)]:
                nc.sync.dma_start(out=dbg_d[name][:], in_=t[:])

        last_ops = []

        def head(r):
            z = psro.tile([8, bc], F32, tag="roz")
            nc.tensor.matmul(z, w_sb["ro_fc1"], h2e, start=True, stop=True)
            zs = work.tile([8, bc], F32, tag="zs")
            nc.scalar.mul(zs, z, 0.2)
            nc.vector.tensor_max(zt[0:8], z, zs)  # leaky relu 0.2
            o_ps = psro.tile([8, bc], F32, tag="roo")
            nc.tensor.matmul(o_ps, w_sb["ro_fc2"], zt, start=True, stop=True)
            last_ops[:] = [o_ps]
            nc.scalar.copy(out_sb[:, r, :], o_ps[0:1])
            if r <= fut - 1:
                nc.scalar.copy(ot, o_ps[0:1])

        def ro_cell(mms, rc, h_out):
            gr = psro.tile([16, 4, bc], F32, tag="rog")
            for k in range(4):
                for i, (lhsT, rhs) in enumerate(mms):
                    nc.tensor.matmul(gr[:, k, :], lhsT[:, 16 * k:16 * k + 16], rhs,
                                     start=(i == 0), stop=(i == len(mms) - 1))
            sifr = work.tile([16, 3, bc], F32, tag="sifr")
            nc.scalar.activation(sifr, gr[:, 0:3, :], AF.Sigmoid)
            tgr = work.tile([16, bc], F32, tag="tgr")
            nc.scalar.activation(tgr, gr[:, 3, :], AF.Tanh)
            a1 = work.tile([16, bc], F32, tag="a1")
            a2 = work.tile([16, bc], F32, tag="a2")
            tcr = work.tile([16, bc], F32, tag="tcr")
            nc.vector.tensor_mul(a1, sifr[:, 1, :], rc)
            nc.vector.tensor_mul(a2, sifr[:, 0, :], tgr)
            nc.vector.tensor_add(rc, a1, a2)
            nc.scalar.activation(tcr, rc, AF.Tanh)
            nc.vector.tensor_mul(h_out, sifr[:, 2, :], tcr)

        head(0)
        for r in range(fut):
            ro_cell([(w_sb["ro1x"], ot), (w_sb["ro1h"], h1e)], rc1, h1e[0:16])
            ro_cell([(w_sb["ro2h1"], h1e), (w_sb["ro2h2"], h2e[0:16])], rc2, h2e[0:16])
            head(r + 1)

        if dbg == 2:
            for name, t in [("d_h1e", h1e), ("d_h2e", h2e), ("d_rc1", rc1), ("d_rc2", rc2)]:
                nc.sync.dma_start(out=dbg_d[name][:], in_=t[:])
        if dbg == 3:
            nc.sync.dma_start(out=dbg_d["d_zt"][:], in_=zt[:])
            ops_sb = states.tile([8, bc], F32, tag="ops_sb")
            nc.scalar.copy(ops_sb, last_ops[0])
            nc.sync.dma_start(out=dbg_d["d_ops"][:], in_=ops_sb[:])
            nc.sync.dma_start(out=dbg_d["d_osb"][:], in_=out_sb[:])
            z2 = psro.tile([8, bc], F32, tag="roz")
            nc.tensor.matmul(z2, w_sb["ro_fc1"], h2e, start=True, stop=True)
            z2s = states.tile([8, bc], F32, tag="z2s")
            nc.scalar.copy(z2s, z2)
            nc.sync.dma_start(out=dbg_d["d_z"][:], in_=z2s[:])

        # keep the partition dim in the AP — integer-indexing it away breaks
        # Tile's subtile dependency tracking (the DMA then reads stale data)
        nc.sync.dma_start(out=out_d[:].rearrange("(o f) b -> o f b", o=1), in_=out_sb)

        for p_ in (psro, psum, xst, work, states, consts):
            p_.release()

    if not nc.is_finalized():
        nc.finalize()
    return nc


_CACHED = {}


def _get_nc(t_steps, fut, bc, loop_steps=None):
    key = (t_steps, fut, bc, loop_steps)
    if key not in _CACHED:
        _CACHED[key] = _build(t_steps, fut, bc, loop_steps=loop_steps)
    return _CACHED[key]


def kernel(x, W_ih1, W_hh1, b_ih1, b_hh1, W_ih2, W_hh2, b_ih2, b_hh2,
           fc1_w, fc1_b, fc2_w, fc2_b, future, _t_steps=None, _trace=False,
           _loop_steps=None):
    x = np.asarray(x, np.float32)
    fut = int(future)
    t_steps = int(_t_steps or x.shape[1])
    bc = x.shape[0] // NCORES

    w = _pack_weights(np.asarray(W_ih1, np.float32), np.asarray(W_hh1, np.float32),
                      np.asarray(b_ih1, np.float32), np.asarray(b_hh1, np.float32),
                      np.asarray(W_ih2, np.float32), np.asarray(W_hh2, np.float32),
                      np.asarray(b_ih2, np.float32), np.asarray(b_hh2, np.float32),
                      np.asarray(fc1_w, np.float32), np.asarray(fc1_b, np.float32),
                      np.asarray(fc2_w, np.float32), np.asarray(fc2_b, np.float32))

    nc = _get_nc(t_steps, fut, bc, _loop_steps)
    in_maps = []
    for c in range(NCORES):
        m = dict(w)
        m["xe"] = _pack_x(x[c * bc : (c + 1) * bc, :t_steps], t_steps)
        in_maps.append(m)

    res = run_bass_kernel_spmd(nc, in_maps, core_ids=list(range(NCORES)), trace=_trace)
    outs = [res.results[c]["out"] for c in range(NCORES)]  # each [fut+1, bc]
    full = np.concatenate(outs, axis=1).T  # [B, fut+1]
    kernel._last_exec_ns = res.exec_time_ns
    return np.ascontiguousarray(full.astype(np.float32))


# revision 20
# speedup vs baseline: 92.8085x; 1.3791x over previous
"""Trainium2 Bass kernel for 2-layer LSTM (H=16) time-series predictor.

Model (reference): x:[B,T] -> per-t scalar input into LSTMCell1(1->16) ->
LSTMCell2(16->16), teacher-forced over T steps, then head(h2)=fc2(leaky(fc1(h2)))
produces out[:,0]; 32 autoregressive steps feed head output back as input.
Output [B, 33].

Sharding: data-parallel over batch across 8 cores (256 batch each), weights
replicated. Per-core layout: hidden dim on partitions, batch on the free dim.

The T-step recurrence runs in a HARDWARE loop (tc.For_i, UNROLL sub-steps per
iteration): a fully unrolled 2048-step body is ~33k instructions and both NEFF
load time and instruction streaming scale with that; the hw loop keeps the
NEFF ~1k instructions regardless of T.

Measured per-instruction costs on this part (wall-clock 10x-loop contrast):
matmul ~1.8us FIXED (N-independent), ACT ~1.4us, DVE ~0.6us, DMA ~free.
So the step is designed for MINIMUM instruction count (8/step):

  1 matmul   gates for BOTH layers in one [128,bc] PSUM tile: partition
             blocks i|f|o|g, each [l1(16)|l2(16)]; rhs = rotating hx tile
             [h1/2(16); h2/2(16); x(1); 1(1)] (K=34).
  1 ACT      sigmoid over all 128 gate partitions; tanh(z) is computed as
             2*sig(2z)-1 with the 2z folded into the g-gate weights.
  5 DVE      tg=2*sig_g-1; m1=sig_f*c; m2=sig_i*tg; c=m1+m2;
             h/2=(sig(2c)-0.5)*sig_o   (+1 more ACT for sig(2c)).

Legality notes (verified on this HW): a 2-input DVE op may read at most ONE
input from PSUM (then partition bases may differ); both-SBUF inputs need
equal bases. Single-input DVE ops can cross partition quadrants freely.
h is stored halved (h/2) so tanh(c)=2*sig(2c)-1 needs no extra affine; the
doubling is folded into the h-rows of the gate weights, and un-done when
snapshotting h for the rollout.
"""

import numpy as np

import concourse.bass as bass
import concourse.tile as tile
from concourse import bacc, mybir
from concourse.bass import ds
from concourse.bass_utils import run_bass_kernel_spmd

F32 = mybir.dt.float32
AF = mybir.ActivationFunctionType
ALU = mybir.AluOpType

H = 16
B = 2048
T = 2048
FUT = 32
NCORES = 8
BC = B // NCORES  # 256 batch per core
NHX = 4           # rotation depth of the hx rhs buffers (x-DMA prefetch depth)
UNROLL = 16       # recurrence steps per hw-loop iteration

# torch gate row order in the 4H weight matrices: i, f, g, o
_G = {"i": slice(0, H), "f": slice(H, 2 * H), "g": slice(2 * H, 3 * H), "o": slice(3 * H, 4 * H)}
# gate order along the 128 PSUM partitions (32 each)
_ORDER = ["i", "f", "o", "g"]


def _pack_weights(W_ih1, W_hh1, b_ih1, b_hh1, W_ih2, W_hh2, b_ih2, b_hh2,
                  fc1_w, fc1_b, fc2_w, fc2_b):
    b1 = b_ih1 + b_hh1  # [64]
    b2 = b_ih2 + b_hh2

    # main-loop lhsT [34, 128]: rows = rhs layout [h1/2(16); h2/2(16); x; 1],
    # cols = gate partition blocks. h-rows x2 (h stored halved); g-gate cols
    # x2 on top (tanh(z) = 2*sig(2z)-1).
    wmall = np.zeros((34, 128), np.float32)
    for k, gn in enumerate(_ORDER):
        c0 = 32 * k
        gs = 2.0 if gn == "g" else 1.0
        wmall[0:16, c0:c0 + 16] = 2.0 * gs * W_hh1[_G[gn], :].T      # h1 -> l1
        wmall[0:16, c0 + 16:c0 + 32] = 2.0 * gs * W_ih2[_G[gn], :].T  # h1 -> l2
        wmall[16:32, c0 + 16:c0 + 32] = 2.0 * gs * W_hh2[_G[gn], :].T  # h2 -> l2
        wmall[32, c0:c0 + 16] = gs * W_ih1[_G[gn], 0]                # x -> l1
        wmall[33, c0:c0 + 16] = gs * b1[_G[gn]]
        wmall[33, c0 + 16:c0 + 32] = gs * b2[_G[gn]]

    # rollout layer1: split into K=1 part (input o) and K=17 part (h1 + bias)
    ro1x = np.zeros((1, 64), np.float32)   # columns: 4 gate blocks of 16
    ro1h = np.zeros((17, 64), np.float32)
    ro2h1 = np.zeros((17, 64), np.float32)  # h1 + bias part of layer2
    ro2h2 = np.zeros((16, 64), np.float32)  # h2 part of layer2
    for k, gn in enumerate(_ORDER):
        c0 = 16 * k
        ro1x[0, c0:c0 + 16] = W_ih1[_G[gn], 0]
        ro1h[0:16, c0:c0 + 16] = W_hh1[_G[gn], :].T
        ro1h[16, c0:c0 + 16] = b1[_G[gn]]
        ro2h1[0:16, c0:c0 + 16] = W_ih2[_G[gn], :].T
        ro2h1[16, c0:c0 + 16] = b2[_G[gn]]
        ro2h2[0:16, c0:c0 + 16] = W_hh2[_G[gn], :].T

    ro_fc1 = np.zeros((17, 8), np.float32)
    ro_fc1[0:16] = fc1_w.T  # fc1_w [8,16]
    ro_fc1[16] = fc1_b

    # M=1 matmuls misbehave on HW — replicate the fc2 column into M=8 and
    # read row 0 of the result instead
    ro_fc2 = np.zeros((9, 8), np.float32)
    ro_fc2[0:8] = fc2_w.T  # fc2_w [1,8]
    ro_fc2[8] = fc2_b

    return dict(wmall=wmall, ro1x=ro1x, ro1h=ro1h,
                ro2h1=ro2h1, ro2h2=ro2h2, ro_fc1=ro_fc1, ro_fc2=ro_fc2)


def _pack_x(x_core, t_steps):
    """x_core [BC, t] -> [t+1, 2, BC]: per step a [x_t; 1] pair (last x row 0)."""
    xe = np.ones((t_steps + 1, 2, x_core.shape[0]), np.float32)
    xe[:, 0, :] = 0.0
    xe[:t_steps, 0, :] = x_core.T.astype(np.float32)
    return xe


# ---------------------------------------------------------------------------
# device kernel
# ---------------------------------------------------------------------------

def _build(t_steps=T, fut=FUT, bc=BC, dbg=False, loop_steps=None,
           unroll=UNROLL, repeat_loop=1):
    # loop_steps < t_steps runs fewer recurrence steps with identical I/O
    # sizes; repeat_loop > 1 re-runs the hw loop (both measurement-only:
    # output is mathematically meaningless in those modes)
    nc = bacc.Bacc("TRN2", target_bir_lowering=False)

    xe_d = nc.dram_tensor("xe", [t_steps + 1, 2, bc], F32, kind="ExternalInput")
    w_d = {}
    for name, shape in [("wmall", [34, 128]),
                        ("ro1x", [1, 64]), ("ro1h", [17, 64]),
                        ("ro2h1", [17, 64]), ("ro2h2", [16, 64]),
                        ("ro_fc1", [17, 8]), ("ro_fc2", [9, 8])]:
        w_d[name] = nc.dram_tensor(name, shape, F32, kind="ExternalInput")
    out_d = nc.dram_tensor("out", [fut + 1, bc], F32, kind="ExternalOutput")

    with tile.TileContext(nc) as tc:
        consts = tc.alloc_tile_pool(name="consts", bufs=1)
        states = tc.alloc_tile_pool(name="states", bufs=1)
        work = tc.alloc_tile_pool(name="work", bufs=3)
        # main psum and rollout psum stay disjoint for the whole kernel
        # (PE-write/engine-read same-bank hazards when recycling banks)
        psum = tc.alloc_tile_pool(name="psum", bufs=2, space="PSUM")
        psro = tc.alloc_tile_pool(name="psro", bufs=1, space="PSUM")

        w_sb = {}
        for name, t_d in w_d.items():
            w_sb[name] = consts.tile(list(t_d.shape), F32, tag=name, name=name)
            nc.sync.dma_start(out=w_sb[name], in_=t_d[:])
        wmall = w_sb["wmall"]

        # rotating rhs buffers: [h1/2(0:16); h2/2(16:32); x(32); ones(33)]
        hx = []
        for q in range(NHX):
            hq = states.tile([34, bc], F32, tag=f"hx{q}", name=f"hx{q}")
            nc.vector.memset(hq, 0.0)
            hx.append(hq)
        cc = states.tile([32, bc], F32, tag="cc")   # [c1; c2] (real scale)
        nc.vector.memset(cc, 0.0)

        # rollout state tiles (layer-1 snapshots are taken between main-loop
        # steps T-1 and T)
        h1e = states.tile([17, bc], F32, tag="h1e")  # h1 | ones
        h2e = states.tile([17, bc], F32, tag="h2e")  # h2 | ones
        rc1 = states.tile([16, bc], F32, tag="rc1")
        rc2 = states.tile([16, bc], F32, tag="rc2")

        # ---------------- main teacher-forced loop ----------------
        # step j computes layer1(t=j) and layer2(t=j-1) from one matmul.
        def body(j_ap, parity, first=False):
            cur = hx[parity % NHX]
            nxt = hx[(parity + 1) % NHX]
            # fill x/ones rows of this step's rhs (prefetched NHX-1 steps of
            # slack: the WAR dependency is on the matmul of step j-NHX)
            if isinstance(j_ap, int):
                nc.sync.dma_start(out=cur[32:34, :], in_=xe_d[j_ap])
            else:
                nc.sync.dma_start(
                    out=cur[32:34, :],
                    in_=xe_d[ds(j_ap, 1)].rearrange("o p b -> (o p) b"))

            g = psum.tile([128, bc], F32, tag="g")
            nc.tensor.matmul(g, wmall, cur, start=True, stop=True)
            sg = psum.tile([128, bc], F32, tag="sg")
            nc.scalar.activation(sg, g, AF.Sigmoid)

            # first step: layer-1 half only (layer-2 gates are not yet valid);
            # gate block k sits at partitions 32k..32k+32 as [l1(16)|l2(16)].
            w16 = 16 if first else 32
            tg = work.tile([32, bc], F32, tag="tg")
            m1 = work.tile([32, bc], F32, tag="m1")
            m2 = work.tile([32, bc], F32, tag="m2")
            sc = work.tile([32, bc], F32, tag="sc")
            # tg = tanh(g_pre) = 2*sig(2*g_pre) - 1 (the 2x is in the weights)
            nc.vector.tensor_scalar(out=tg[0:w16], in0=sg[96:96 + w16, :],
                                    scalar1=2.0, scalar2=1.0,
                                    op0=ALU.mult, op1=ALU.subtract)
            nc.vector.tensor_mul(m1[0:w16], sg[32:32 + w16, :], cc[0:w16])
            nc.vector.tensor_mul(m2[0:w16], sg[0:w16, :], tg[0:w16])
            nc.vector.tensor_add(cc[0:w16], m1[0:w16], m2[0:w16])
            # h/2 = tanh(c)/2 * sig_o = (sig(2c) - 0.5) * sig_o
            nc.scalar.activation(sc[0:w16], cc[0:w16], AF.Sigmoid, scale=2.0)
            nc.vector.scalar_tensor_tensor(out=nxt[0:w16], in0=sc[0:w16],
                                           scalar=0.5, in1=sg[64:64 + w16, :],
                                           op0=ALU.subtract, op1=ALU.mult)

        n_loop = t_steps if loop_steps is None else loop_steps
        assert n_loop >= unroll and (n_loop - unroll) % unroll == 0, n_loop
        # peel the first `unroll` steps (covers the j==0 special case)
        for j in range(unroll):
            body(j, j % NHX, first=(j == 0))
        # hardware loop over the middle steps, `unroll` at a time
        if n_loop > unroll:
            for _rep in range(repeat_loop):
                with tc.For_i(unroll, n_loop, unroll) as jb:
                    for u in range(unroll):
                        body(jb + u, u % NHX)
        # snapshot layer-1 state before the final (layer-2-only) step clobbers
        # it; un-halve h on the way out
        nc.vector.tensor_scalar(out=h1e[0:16], in0=hx[n_loop % NHX][0:16, :],
                                scalar1=2.0, scalar2=None, op0=ALU.mult)
        nc.scalar.copy(rc1, cc[0:16])                        # c1(T-1), real
        body(n_loop, n_loop % NHX)

        # ---------------- rollout ----------------
        ot = states.tile([1, bc], F32, tag="ot")     # current head output
        zt = states.tile([9, bc], F32, tag="zt")     # leaky(fc1) | ones
        out_sb = states.tile([1, fut + 1, bc], F32, tag="out_sb")
        # ones rows (memset can't start at partition 16/8 — DMA from xe ones row)
        nc.sync.dma_start(out=h1e[16:17, :], in_=xe_d[n_loop, 1:2])
        nc.sync.dma_start(out=h2e[16:17, :], in_=xe_d[n_loop, 1:2])
        nc.sync.dma_start(out=zt[8:9, :], in_=xe_d[n_loop, 1:2])

        # h2(T-1): repartition via DMA (hx holds h/2 -> x2 in place after)
        nc.sync.dma_start(out=h2e[0:16, :], in_=hx[(n_loop + 1) % NHX][16:32, :])
        nc.vector.tensor_scalar(out=h2e[0:16, :], in0=h2e[0:16, :],
                                scalar1=2.0, scalar2=None, op0=ALU.mult)
        nc.sync.dma_start(out=rc2[:], in_=cc[16:32, :])

        def head(r):
            z = psro.tile([8, bc], F32, tag="roz")
            nc.tensor.matmul(z, w_sb["ro_fc1"], h2e, start=True, stop=True)
            zs = work.tile([8, bc], F32, tag="zs")
            nc.scalar.mul(zs, z, 0.2)
            nc.vector.tensor_max(zt[0:8], z, zs)  # leaky relu 0.2
            o_ps = psro.tile([8, bc], F32, tag="roo")
            nc.tensor.matmul(o_ps, w_sb["ro_fc2"], zt, start=True, stop=True)
            nc.scalar.copy(out_sb[:, r, :], o_ps[0:1])
            if r <= fut - 1:
                nc.scalar.copy(ot, o_ps[0:1])

        def ro_cell(mms, rc, h_out):
            gr = psro.tile([16, 4, bc], F32, tag="rog")
            for k in range(4):
                for i, (lhsT, rhs) in enumerate(mms):
                    nc.tensor.matmul(gr[:, k, :], lhsT[:, 16 * k:16 * k + 16], rhs,
                                     start=(i == 0), stop=(i == len(mms) - 1))
            sifr = work.tile([16, 3, bc], F32, tag="sifr")
            nc.scalar.activation(sifr, gr[:, 0:3, :], AF.Sigmoid)
            tgr = work.tile([16, bc], F32, tag="tgr")
            nc.scalar.activation(tgr, gr[:, 3, :], AF.Tanh)
            a1 = work.tile([16, bc], F32, tag="a1")
            a2 = work.tile([16, bc], F32, tag="a2")
            tcr = work.tile([16, bc], F32, tag="tcr")
            nc.vector.tensor_mul(a1, sifr[:, 1, :], rc)
            nc.vector.tensor_mul(a2, sifr[:, 0, :], tgr)
            nc.vector.tensor_add(rc, a1, a2)
            nc.scalar.activation(tcr, rc, AF.Tanh)
            nc.vector.tensor_mul(h_out, sifr[:, 2, :], tcr)

        head(0)
        for r in range(fut):
            ro_cell([(w_sb["ro1x"], ot), (w_sb["ro1h"], h1e)], rc1, h1e[0:16])
            ro_cell([(w_sb["ro2h1"], h1e), (w_sb["ro2h2"], h2e[0:16])], rc2, h2e[0:16])
            head(r + 1)

        # keep the partition dim in the AP — integer-indexing it away breaks
        # Tile's subtile dependency tracking (the DMA then reads stale data)
        nc.sync.dma_start(out=out_d[:].rearrange("(o f) b -> o f b", o=1), in_=out_sb)

        for p_ in (psro, psum, work, states, consts):
            p_.release()

    if not nc.is_finalized():
        nc.finalize()
    return nc


_CACHED = {}


def _get_nc(t_steps, fut, bc, loop_steps=None):
    key = (t_steps, fut, bc, loop_steps)
    if key not in _CACHED:
        _CACHED[key] = _build(t_steps, fut, bc, loop_steps=loop_steps)
    return _CACHED[key]


def kernel(x, W_ih1, W_hh1, b_ih1, b_hh1, W_ih2, W_hh2, b_ih2, b_hh2,
           fc1_w, fc1_b, fc2_w, fc2_b, future, _t_steps=None, _trace=False,
           _loop_steps=None):
    x = np.asarray(x, np.float32)
    fut = int(future)
    t_steps = int(_t_steps or x.shape[1])
    bc = x.shape[0] // NCORES

    w = _pack_weights(np.asarray(W_ih1, np.float32), np.asarray(W_hh1, np.float32),
                      np.asarray(b_ih1, np.float32), np.asarray(b_hh1, np.float32),
                      np.asarray(W_ih2, np.float32), np.asarray(W_hh2, np.float32),
                      np.asarray(b_ih2, np.float32), np.asarray(b_hh2, np.float32),
                      np.asarray(fc1_w, np.float32), np.asarray(fc1_b, np.float32),
                      np.asarray(fc2_w, np.float32), np.asarray(fc2_b, np.float32))

    nc = _get_nc(t_steps, fut, bc, _loop_steps)
    in_maps = []
    for c in range(NCORES):
        m = dict(w)
        m["xe"] = _pack_x(x[c * bc : (c + 1) * bc, :t_steps], t_steps)
        in_maps.append(m)

    res = run_bass_kernel_spmd(nc, in_maps, core_ids=list(range(NCORES)), trace=_trace)
    outs = [res.results[c]["out"] for c in range(NCORES)]  # each [fut+1, bc]
    full = np.concatenate(outs, axis=1).T  # [B, fut+1]
    kernel._last_exec_ns = res.exec_time_ns
    return np.ascontiguousarray(full.astype(np.float32))
